# revision 11
# baseline (speedup 1.0000x reference)
"""AttentionBlock (GroupNorm + 8-head self-attention + proj residual) on 8 trn2 cores.

Sharding: core c handles batch b = c//4 and head pair p = c%4 (heads 2p, 2p+1).
Each core computes GroupNorm(x[b]) (duplicated within the batch group), its two
heads' q/k/v projections, full 4096x4096 attention for those heads, and the
partial output projection proj_w[:, 128p:128p+128] @ attn_out. The four partials
per batch are summed on the host along with the residual x and proj_b.

All matmuls run in fp16 (fp22 multiply / fp32 accumulate on the PE), which keeps
the tensor engine at 1 cycle/row. Scores are exponentiated on the scalar engine
(PSUM -> SBUF) with the 1/8 scale and a -4 shift folded into the activation;
softmax denominators come for free as a 65th "ones" column on the transposed V.
"""

import numpy as np

B, C, H, W = 2, 512, 64, 64
NUM_HEADS = 8
GROUPS = 32
EPS = 1e-5
N = H * W          # 4096 pixels
D = 64             # head dim
NB = 8             # 512-wide pixel blocks
CT = 4             # 128-row channel tiles
KT = 32            # 128-wide key tiles
SHIFT = 4.0        # logit shift inside exp (cancels in softmax)
SCALE = D ** -0.5  # 1/8

_cache = {}


def _build():
    import concourse.bass as bass
    import concourse.bacc as bacc
    import concourse.tile as tile
    import concourse.mybir as mybir

    F32 = mybir.dt.float32
    F16 = mybir.dt.float16
    AF = mybir.ActivationFunctionType
    OP = mybir.AluOpType

    nc = bacc.Bacc("TRN2", target_bir_lowering=False, debug=False, num_devices=8)

    x_b = nc.declare_dram_parameter("x_b", [C, N], F32, isOutput=False)
    wq_t = nc.declare_dram_parameter("wq_t", [C, 128], F16, isOutput=False)
    wk_t = nc.declare_dram_parameter("wk_t", [C, 128], F16, isOutput=False)
    wv_t = nc.declare_dram_parameter("wv_t", [C, 128], F16, isOutput=False)
    wp2 = nc.declare_dram_parameter("wp2", [64, 2, C], F16, isOutput=False)
    b_qkv = nc.declare_dram_parameter("b_qkv", [128, 3], F32, isOutput=False)
    gnw = nc.declare_dram_parameter("gnw", [128, CT], F32, isOutput=False)
    gnb = nc.declare_dram_parameter("gnb", [128, CT], F32, isOutput=False)
    ind_g = nc.declare_dram_parameter("ind_g", [128, CT, 32], F16, isOutput=False)
    ind_c = nc.declare_dram_parameter("ind_c", [32, CT, 128], F16, isOutput=False)
    ident = nc.declare_dram_parameter("ident", [128, 128], F16, isOutput=False)
    partial = nc.declare_dram_parameter("partial", [C, N], F32, isOutput=True)
    recip_dram = nc.dram_tensor("recip_dram", [2, N], F16)

    with tile.TileContext(nc) as tc:
        with tc.tile_pool(name="persist", bufs=1) as pp:

            # ---------- load constants / weights ----------
            wq_sb = pp.tile([128, CT, 128], F16, tag="wq")
            wk_sb = pp.tile([128, CT, 128], F16, tag="wk")
            wv_sb = pp.tile([128, CT, 128], F16, tag="wv")
            nc.sync.dma_start(out=wq_sb, in_=wq_t.rearrange("(c p) m -> p c m", p=128))
            nc.sync.dma_start(out=wk_sb, in_=wk_t.rearrange("(c p) m -> p c m", p=128))
            nc.sync.dma_start(out=wv_sb, in_=wv_t.rearrange("(c p) m -> p c m", p=128))
            wp_sb = pp.tile([64, 2, C], F16, tag="wp")
            nc.sync.dma_start(out=wp_sb, in_=wp2[:, :, :])
            bqkv_sb = pp.tile([128, 3], F32, tag="bqkv")
            nc.sync.dma_start(out=bqkv_sb, in_=b_qkv[:, :])
            gnw_sb = pp.tile([128, CT], F32, tag="gnw")
            gnb_sb = pp.tile([128, CT], F32, tag="gnb")
            nc.sync.dma_start(out=gnw_sb, in_=gnw[:, :])
            nc.sync.dma_start(out=gnb_sb, in_=gnb[:, :])
            indg_sb = pp.tile([128, CT, 32], F16, tag="indg")
            nc.sync.dma_start(out=indg_sb, in_=ind_g[:, :, :])
            indc_sb = pp.tile([32, CT, 128], F16, tag="indc")
            nc.sync.dma_start(out=indc_sb, in_=ind_c[:, :, :])
            id_sb = pp.tile([128, 128], F16, tag="ident")
            nc.sync.dma_start(out=id_sb, in_=ident[:, :])

            h_sb = pp.tile([128, CT, N], F16, tag="h")

            # ---------- GroupNorm ----------
            with tc.tile_pool(name="xload", bufs=1) as xp, \
                 tc.tile_pool(name="gn_sb", bufs=1) as gp, \
                 tc.tile_pool(name="gn_ps", bufs=1, space="PSUM") as gps, \
                 tc.tile_pool(name="gn_ps2", bufs=2, space="PSUM") as gps2:
                x_sb = xp.tile([128, CT, N], F32, tag="x")
                for t in range(CT):
                    nc.sync.dma_start(
                        out=x_sb[:, t, :], in_=x_b[128 * t:128 * (t + 1), :]
                    )
                bst = gp.tile([128, CT, 8, 6], F32, tag="bst")
                mv = gp.tile([128, CT, 2], F32, tag="mv")
                t2 = gp.tile([128, CT, 2], F16, tag="t2")
                mm2 = gp.tile([128, CT], F32, tag="mm2")
                for t in range(CT):
                    for s in range(8):
                        nc.vector.bn_stats(
                            out=bst[:, t, s, :],
                            in_=x_sb[:, t, 512 * s:512 * (s + 1)],
                        )
                    nc.vector.bn_aggr(out=mv[:, t, :], in_=bst[:, t, :, :])
                    # t2 = (mean, var + mean^2) in f16 for the group-sum matmul
                    nc.vector.tensor_copy(t2[:, t, 0:1], mv[:, t, 0:1])
                    nc.vector.tensor_mul(mm2[:, t:t + 1], mv[:, t, 0:1], mv[:, t, 0:1])
                    nc.vector.tensor_add(t2[:, t, 1:2], mv[:, t, 1:2], mm2[:, t:t + 1])
                gsum = gps.tile([32, 2], F32, tag="gsum")
                for t in range(CT):
                    nc.tensor.matmul(
                        gsum, indg_sb[:, t, :], t2[:, t, :],
                        start=(t == 0), stop=(t == CT - 1),
                    )
                gm = gp.tile([32, 2], F32, tag="gm")
                nc.vector.tensor_scalar_mul(gm, gsum, 1.0 / 16.0)
                mg2 = gp.tile([32, 1], F32, tag="mg2")
                nc.vector.tensor_mul(mg2, gm[:, 0:1], gm[:, 0:1])
                vg = gp.tile([32, 1], F32, tag="vg")
                nc.vector.tensor_tensor(out=vg, in0=gm[:, 1:2], in1=mg2, op=OP.subtract)
                # rstd = 1 / sqrt(var + eps)
                eps_sb = gp.tile([32, 1], F32, tag="eps")
                nc.vector.memset(eps_sb, EPS)
                sdv = gp.tile([32, 1], F32, tag="sdv")
                nc.scalar.activation(out=sdv, in_=vg, func=AF.Sqrt, bias=eps_sb)
                rstd = gp.tile([32, 1], F32, tag="rstd")
                nc.vector.reciprocal(out=rstd, in_=sdv)
                gstat = gp.tile([32, 2], F16, tag="gstat")
                nc.vector.tensor_copy(gstat[:, 0:1], gm[:, 0:1])
                nc.vector.tensor_copy(gstat[:, 1:2], rstd)
                scl = gp.tile([128, CT], F32, tag="scl")
                bia = gp.tile([128, CT], F32, tag="bia")
                tmp = gp.tile([128, CT], F32, tag="tmpgn")
                for t in range(CT):
                    cb = gps2.tile([128, 2], F32, tag="cb")
                    nc.tensor.matmul(cb, indc_sb[:, t, :], gstat, start=True, stop=True)
                    # scale = rstd * gn_w ; bias = gn_b - mean * scale
                    nc.vector.tensor_mul(scl[:, t:t + 1], cb[:, 1:2], gnw_sb[:, t:t + 1])
                    nc.vector.tensor_mul(tmp[:, t:t + 1], cb[:, 0:1], scl[:, t:t + 1])
                    nc.vector.tensor_tensor(
                        out=bia[:, t:t + 1], in0=gnb_sb[:, t:t + 1],
                        in1=tmp[:, t:t + 1], op=OP.subtract,
                    )
                for t in range(CT):
                    nc.vector.tensor_scalar(
                        out=h_sb[:, t, :], in0=x_sb[:, t, :],
                        scalar1=scl[:, t:t + 1], scalar2=bia[:, t:t + 1],
                        op0=OP.mult, op1=OP.add,
                    )

            # ---------- qkv projections ----------
            mp_cm = tc.tile_pool(name="mainphase", bufs=1)
            mp = mp_cm.__enter__()
            q_sb = mp.tile([128, N], F16, tag="q")
            k_sb = mp.tile([128, N], F16, tag="k")
            v_sb = mp.tile([128, N], F16, tag="v")
            with tc.tile_pool(name="qkv_ps", bufs=3, space="PSUM") as qps:
                for nb in range(NB):
                    sl = slice(512 * nb, 512 * (nb + 1))
                    for wi, (w_sb, dst) in enumerate(
                        [(wq_sb, q_sb), (wk_sb, k_sb), (wv_sb, v_sb)]
                    ):
                        ps = qps.tile([128, 512], F32, tag="qkvps")
                        for cc in range(CT):
                            nc.tensor.matmul(
                                ps, w_sb[:, cc, :], h_sb[:, cc, sl],
                                start=(cc == 0), stop=(cc == CT - 1),
                            )
                        nc.vector.tensor_scalar_add(
                            out=dst[:, sl], in0=ps, scalar1=bqkv_sb[:, wi:wi + 1]
                        )

            # ---------- transpose v (and plant the ones column) ----------
            # vt layout: [key_within_tile, pair, ktile, 65] ; column 64 == 1.0
            vt_sb = mp.tile([128, 2, KT, 65], F16, tag="vt")
            nc.vector.memset(vt_sb, 1.0)
            with tc.tile_pool(name="tp_ps", bufs=2, space="PSUM") as tps:
                for kt in range(KT):
                    tp = tps.tile([128, 128], F16, tag="tp")
                    nc.tensor.transpose(tp, v_sb[:, 128 * kt:128 * (kt + 1)], id_sb)
                    nc.vector.tensor_copy(
                        out=vt_sb[:, :, kt, 0:64],
                        in_=tp.rearrange("p (h d) -> p h d", h=2),
                    )

            # ---------- attention ----------
            attn = [
                mp.tile([64, N], F16, tag="attnA", name="attnA"),
                mp.tile([64, N], F16, tag="attnB", name="attnB"),
            ]
            sums_tmp = mp.tile([65, 2, N], F32, tag="sums_tmp")
            shift_sb = mp.tile([128, 1], F32, tag="shift")
            nc.vector.memset(shift_sb, -SHIFT)
            with tc.tile_pool(name="s_ps", bufs=1, space="PSUM") as sps, \
                 tc.tile_pool(name="av_ps", bufs=2, space="PSUM") as aps, \
                 tc.tile_pool(name="p_sb", bufs=4) as psb:
                for qb in range(NB):
                    qsl = slice(512 * qb, 512 * (qb + 1))
                    av = [
                        aps.tile([65, 512], F32, tag="avA", name="avA"),
                        aps.tile([65, 512], F32, tag="avB", name="avB"),
                    ]
                    for g in range(KT // 2):
                        s_t = [
                            sps.tile([128, 1024], F32, tag="sA", name="sA"),
                            sps.tile([128, 1024], F32, tag="sB", name="sB"),
                        ]
                        for pr in range(2):
                            rows = slice(64 * pr, 64 * (pr + 1))
                            for sub in range(2):
                                kt = 2 * g + sub
                                nc.tensor.matmul(
                                    s_t[pr][:, 512 * sub:512 * (sub + 1)],
                                    k_sb[rows, 128 * kt:128 * (kt + 1)],
                                    q_sb[rows, qsl],
                                    start=True, stop=True,
                                    tile_position=(64 * pr, 0),
                                )
                        p_t = [
                            psb.tile([128, 1024], F16, tag="pA", name="pA"),
                            psb.tile([128, 1024], F16, tag="pB", name="pB"),
                        ]
                        for pr in range(2):
                            nc.scalar.activation(
                                out=p_t[pr], in_=s_t[pr], func=AF.Exp,
                                scale=SCALE, bias=shift_sb,
                            )
                        for pr in range(2):
                            for sub in range(2):
                                kt = 2 * g + sub
                                nc.tensor.matmul(
                                    av[pr],
                                    vt_sb[:, pr, kt, :],
                                    p_t[pr][:, 512 * sub:512 * (sub + 1)],
                                    start=(g == 0 and sub == 0),
                                    stop=(g == KT // 2 - 1 and sub == 1),
                                )
                    for pr in range(2):
                        nc.vector.tensor_copy(
                            out=attn[pr][:, qsl], in_=av[pr][0:64, :]
                        )
                        nc.vector.tensor_copy(
                            out=sums_tmp[64:65, pr, qsl], in_=av[pr][64:65, :]
                        )

            # ---------- softmax denominators ----------
            # spread the 8192 sums across 128 partitions for the reciprocal,
            # then gather + cast to a single-partition f16 row and broadcast.
            sums128 = mp.tile([128, 64], F32, tag="sums128")
            nc.sync.dma_start(out=sums128, in_=sums_tmp[64:65, :, :])
            rec128 = mp.tile([128, 64], F32, tag="rec128")
            nc.vector.reciprocal(out=rec128, in_=sums128)
            # bounce through DRAM (cast to f16) so we can broadcast-read it
            # across 64 partitions (partition-step-0 APs need a DRAM source)
            nc.gpsimd.dma_start(out=recip_dram[:, :], in_=rec128)
            bc_sb = mp.tile([64, 2, N], F16, tag="bc")
            for pr in range(2):
                src = recip_dram[pr:pr + 1, :]
                bcast = bass.AP(
                    tensor=src.tensor, offset=src.offset, ap=[[0, 64], [1, N]]
                )
                nc.gpsimd.dma_start(out=bc_sb[:, pr, :], in_=bcast)

            # ---------- normalize + projection ----------
            with tc.tile_pool(name="y_ps", bufs=3, space="PSUM") as yps, \
                 tc.tile_pool(name="y_sb", bufs=3) as ysb:
                for pr in range(2):
                    for nb in range(NB):
                        sl = slice(512 * nb, 512 * (nb + 1))
                        nc.vector.tensor_mul(
                            attn[pr][:, sl], attn[pr][:, sl], bc_sb[:, pr, sl]
                        )
                for oc in range(CT):
                    for nb in range(NB):
                        sl = slice(512 * nb, 512 * (nb + 1))
                        yp = yps.tile([128, 512], F32, tag="yp")
                        for pr in range(2):
                            nc.tensor.matmul(
                                yp, wp_sb[:, pr, 128 * oc:128 * (oc + 1)],
                                attn[pr][:, sl],
                                start=(pr == 0), stop=(pr == 1),
                            )
                        ye = ysb.tile([128, 512], F32, tag="ye")
                        nc.vector.tensor_copy(ye, yp)
                        nc.sync.dma_start(
                            out=partial[128 * oc:128 * (oc + 1), sl], in_=ye
                        )
            mp_cm.__exit__(None, None, None)

    nc.compile()
    return nc


def _get_runner():
    """Compile once and build a cached jitted 8-core runner."""
    if "runner" in _cache:
        return _cache["runner"]

    import jax
    from jax.experimental.shard_map import shard_map
    from jax.sharding import Mesh, PartitionSpec
    from concourse import bass2jax

    nc = _build()
    bass2jax.install_neuronx_cc_hook()

    import concourse.mybir as mybir

    part_name = nc.partition_id_tensor.name if nc.partition_id_tensor else None
    in_names, out_names, out_avals = [], [], []
    for alloc in nc.m.functions[0].allocations:
        if not isinstance(alloc, mybir.MemoryLocationSet):
            continue
        name = alloc.memorylocations[0].name
        if alloc.kind == "ExternalInput":
            if name != part_name:
                in_names.append(name)
        elif alloc.kind == "ExternalOutput":
            out_names.append(name)
            out_avals.append(
                jax.core.ShapedArray(
                    tuple(alloc.tensor_shape), mybir.dt.np(alloc.dtype)
                )
            )
    n_params = len(in_names)
    all_in_names = in_names + out_names
    if part_name is not None:
        all_in_names.append(part_name)

    def _body(*args):
        operands = list(args)
        if part_name is not None:
            operands.append(bass2jax.partition_id_tensor())
        outs = bass2jax._bass_exec_p.bind(
            *operands,
            out_avals=tuple(out_avals),
            in_names=tuple(all_in_names),
            out_names=tuple(out_names),
            lowering_input_output_aliases=(),
            sim_require_finite=True,
            sim_require_nnan=True,
            nc=nc,
        )
        return tuple(outs)

    devices = jax.devices()[:8]
    mesh = Mesh(np.asarray(devices), ("core",))
    n_outs = len(out_names)
    sharded = jax.jit(
        shard_map(
            _body, mesh=mesh,
            in_specs=(PartitionSpec("core"),) * (n_params + n_outs),
            out_specs=(PartitionSpec("core"),) * n_outs,
            check_rep=False,
        ),
        donate_argnums=tuple(range(n_params, n_params + n_outs)),
        keep_unused=True,
    )

    def run(in_maps):
        concat_in = [
            np.concatenate([np.asarray(m[name]) for m in in_maps], axis=0)
            for name in in_names
        ]
        zeros = [
            np.zeros((8 * a.shape[0], *a.shape[1:]), a.dtype) for a in out_avals
        ]
        out = sharded(*concat_in, *zeros)
        return [
            {
                name: np.asarray(out[i]).reshape(8, *out_avals[i].shape)[c]
                for i, name in enumerate(out_names)
            }
            for c in range(8)
        ]

    _cache["runner"] = (run, sharded, in_names, out_avals, out_names)
    return _cache["runner"]


def _host_inputs(x, gn_w, gn_b, qkv_w, qkv_b, proj_w):
    """Build the 8 per-core input maps (host-side shard/layout prep)."""
    x = np.asarray(x, np.float32).reshape(B, C, N)
    qkv_w = np.asarray(qkv_w, np.float32)
    proj_w = np.asarray(proj_w, np.float32)
    gn_w = np.asarray(gn_w, np.float32)
    gn_b = np.asarray(gn_b, np.float32)
    qkv_b = np.asarray(qkv_b, np.float32)

    gnw_t = np.ascontiguousarray(gn_w.reshape(CT, 128).T)
    gnb_t = np.ascontiguousarray(gn_b.reshape(CT, 128).T)

    p_idx = np.arange(128)
    ind_g = np.zeros((128, CT, 32), np.float16)
    ind_c = np.zeros((32, CT, 128), np.float16)
    for t in range(CT):
        g = 8 * t + p_idx // 16
        ind_g[p_idx, t, g] = 1.0
        ind_c[g, t, p_idx] = 1.0
    ident = np.eye(128, dtype=np.float16)

    in_maps = []
    for c in range(8):
        b, p = divmod(c, 4)
        r = slice(128 * p, 128 * (p + 1))
        # wp2[d, h2, o] = proj_w[o, 128p + 64*h2 + d]
        wp2 = np.ascontiguousarray(
            proj_w[:, r].reshape(C, 2, 64).transpose(2, 1, 0).astype(np.float16)
        )
        in_maps.append({
            "x_b": np.ascontiguousarray(x[b]),
            "wq_t": np.ascontiguousarray(qkv_w[r, :].T.astype(np.float16)),
            "wk_t": np.ascontiguousarray(
                qkv_w[C + 128 * p:C + 128 * (p + 1), :].T.astype(np.float16)
            ),
            "wv_t": np.ascontiguousarray(
                qkv_w[2 * C + 128 * p:2 * C + 128 * (p + 1), :].T.astype(np.float16)
            ),
            "wp2": wp2,
            "b_qkv": np.ascontiguousarray(
                np.stack(
                    [qkv_b[r], qkv_b[C + 128 * p:C + 128 * (p + 1)],
                     qkv_b[2 * C + 128 * p:2 * C + 128 * (p + 1)]], axis=1
                ).astype(np.float32)
            ),
            "gnw": gnw_t,
            "gnb": gnb_t,
            "ind_g": ind_g,
            "ind_c": ind_c,
            "ident": ident,
        })
    return in_maps


def kernel(x, gn_w, gn_b, qkv_w, qkv_b, proj_w, proj_b):
    run, *_ = _get_runner()
    in_maps = _host_inputs(x, gn_w, gn_b, qkv_w, qkv_b, proj_w)
    results = run(in_maps)
    x_f = np.asarray(x, np.float32).reshape(B, C, N)
    out = np.empty((B, C, N), np.float32)
    for b in range(B):
        acc = x_f[b] + np.asarray(proj_b, np.float32)[:, None]
        for p in range(4):
            acc = acc + results[4 * b + p]["partial"]
        out[b] = acc
    return out.reshape(B, C, H, W).astype(np.float32)
